# revision 18
# baseline (speedup 1.0000x reference)
"""Routed low-rank FFN (MoE-style) Trainium2 kernel, v7.

out[n] = x[n] @ U[pids[n]] @ V[pids[n]] + bias

Strategy (expert-parallel over 8 NeuronCores):
  - Host: stable-sort tokens by pid; expert p's tokens go to core p // 8.
    Each expert's token list is split into chunks of <= 128 tokens
    ("groups"). Groups are sorted by size (desc) per core and padded to a
    common per-index capacity C_g (max across cores, rounded up to 4), so
    the SPMD program is identical on all cores. Groups are processed in
    PAIRS: lo = even slot (partitions/PSUM rows 0-63), hi = odd slot
    (64-127).
  - Everything crosses HBM in float16 (halves DMA bytes vs f32; end-to-end
    rel err ~1e-3 vs the 2e-2 gate). Every DMA spans the full 128
    partitions so the 16 SBUF AXI ports stay balanced. Loads are emitted
    in first-use order, alternating the two HWDGE rings (sync/scalar).
  - The PE on this part runs at a fixed 1.2 GHz (the HAM clock gate never
    opens - verified with 4.5us of back-to-back warmup matmuls), so both
    stages minimize moving-operand cycles with full 128x128 fp16
    stationary operands (fast weight load):
      * stage 1: stationary U_pair[k] = [U_lo_k | U_hi_k]; h^T accumulates
        in PSUM (lo rows 0-63, hi rows 64-127; the off-half of each
        matmul's output is garbage and ignored). 8 matmuls of N=C_g per
        group.
      * stage 2: stationary V chunk [128, 128] (rows 0-63 = V_lo, rows
        64-127 = V_hi), moving h^T [128, C0+C1] whose off-quadrants are
        zeroed -> out^T chunk [128, C0+C1]. One fused matmul per output
        chunk per PAIR: 8*(C0+C1) cycles, and out^T lands on all 128
        partitions.
  - Epilogue per pair: two DVE tensor_adds (chunks 0-3 / 4-7, pipelined
    against the stage-2 matmuls) add a transposed-broadcast bias and cast
    to fp16; two balanced [128, 4*(C0+C1)] stores per pair.
  - Host: inverse-permute + transpose rows back to token order, cast f32.
"""

import os

import numpy as np

N_CORES = 8
D_IN = 1024
RANK = 64
D_OUT = 1024
KC = 8  # number of 128-deep contraction chunks: D_IN // 128
OC = 8  # number of 128-wide output chunks: D_OUT // 128
MAX_CHUNK = 128  # max tokens per group (PE stationary-col limit)

# Set by kernel() after a traced run (KERNEL_TRACE=1): HW kernel span in ns.
LAST_EXEC_TIME_NS = None
LAST_RESULTS = None

_PROGRAM_CACHE = {}


def _route(pids: np.ndarray, n_experts: int):
    """Group token indices by expert, chunk to MAX_CHUNK, assign to cores.

    Returns per-core list of (expert, token_index_array), sorted by chunk
    size descending so same-index groups across cores have similar sizes
    (and the last pair computed is the smallest -> shortest tail).
    """
    order = np.argsort(pids, kind="stable")
    counts = np.bincount(pids, minlength=n_experts)
    per_core = max(1, n_experts // N_CORES)
    core_groups = [[] for _ in range(N_CORES)]
    off = 0
    for p in range(n_experts):
        toks = order[off : off + counts[p]]
        off += counts[p]
        for s in range(0, len(toks), MAX_CHUNK):
            core_groups[min(p // per_core, N_CORES - 1)].append(
                (p, toks[s : s + MAX_CHUNK])
            )
    for gs in core_groups:
        gs.sort(key=lambda g: -len(g[1]))
    return core_groups


def _plan(core_groups):
    """Static shapes shared by all cores: capacities, offsets, layouts."""
    G = max(len(gs) for gs in core_groups)
    if G % 2:
        G += 1
    C = []
    for g in range(G):
        m = max((len(gs[g][1]) for gs in core_groups if len(gs) > g), default=0)
        C.append(max(8, 4 * -(-m // 4)))
    CP = [C[2 * p] + C[2 * p + 1] for p in range(G // 2)]
    CPmax = max(CP)
    # DRAM free-dim element offsets (per partition), fp16.
    # pair p block: [u_pair (KC*128) | x_lo (KC*C0) | x_hi (KC*C1) | vb (1024)]
    # pair 0 appends the transposed-broadcast bias after its vb block.
    pair_off = []  # (u_o, x_o, vb_o)
    off = 0
    for p in range(G // 2):
        u_o = off
        x_o = u_o + KC * 128
        vb_o = x_o + KC * CP[p]
        off = vb_o + 1024 + (OC * CPmax if p == 0 else 0)
        pair_off.append((u_o, x_o, vb_o))
    # output: od[128, sum_p OC*CP_p]; pair p at column offset ocol[p]
    ocol = np.concatenate([[0], np.cumsum([OC * cp for cp in CP])]).astype(int)
    return {
        "G": G,
        "C": tuple(C),
        "CP": tuple(CP),
        "CPmax": CPmax,
        "pair_off": pair_off,
        "F": off,
        "ocol": ocol,
        "Ftot_out": int(ocol[-1]),
    }


def _pack_core(gs, plan, x16, U16, V16, bias16):
    """Build one core's [128, F] fp16 input blob."""
    G, C, CPmax = plan["G"], plan["C"], plan["CPmax"]
    ind = np.zeros((128, plan["F"]), np.float16)
    for p in range(G // 2):
        u_o, x_o, vb_o = plan["pair_off"][p]
        if p == 0:
            # bias^T broadcast: [p, c, t] = bias[c*128 + p]
            bt = np.broadcast_to(
                bias16.reshape(OC, 128).T[:, :, None], (128, OC, CPmax)
            )
            ind[:, vb_o + 1024 : vb_o + 1024 + OC * CPmax] = bt.reshape(128, -1)
        upair = np.zeros((128, KC, 128), np.float16)
        xoff = x_o
        for half in range(2):
            g = 2 * p + half
            Cg = C[g]
            if g < len(gs):
                e, toks = gs[g]
                # U [1024, 64] -> [k, p, r] -> partition-major [p, k, r]
                upair[:, :, half * 64 : (half + 1) * 64] = (
                    U16[e].reshape(KC, 128, RANK).transpose(1, 0, 2)
                )
                # V [64, 1024] on partition half `half`
                ind[64 * half : 64 * half + 64, vb_o : vb_o + 1024] = V16[e]
                # x block [Cg, 1024] -> [d, t] -> [k, p, t] -> [p, k, t]
                blk = np.zeros((Cg, D_IN), np.float16)
                blk[: len(toks)] = x16[toks]
                ind[:, xoff : xoff + KC * Cg] = (
                    blk.T.reshape(KC, 128, Cg).transpose(1, 0, 2).reshape(128, -1)
                )
            xoff += KC * Cg
        ind[:, u_o : u_o + KC * 128] = upair.reshape(128, -1)
    return ind


def _unpack_core(od, gs, plan, out):
    """Scatter one core's [128, Ftot_out] fp16 result into out[N, D_OUT]."""
    ocol = plan["ocol"]
    C, CP = plan["C"], plan["CP"]
    for g, (e, toks) in enumerate(gs):
        p, half = divmod(g, 2)
        cp = CP[p]
        blk = np.asarray(od[:, ocol[p] : ocol[p] + OC * cp]).reshape(128, OC, cp)
        # blk[p, c, t] = out[t, c*128+p]; hi group's tokens start at column C0
        t0 = C[2 * p] if half else 0
        out[toks] = (
            blk[:, :, t0 : t0 + len(toks)]
            .transpose(2, 1, 0)
            .reshape(len(toks), D_OUT)
            .astype(np.float32)
        )


def _build_program(plan):
    """Build the SPMD Bass/Tile program for one capacity profile."""
    import concourse.tile as tile
    from concourse import bacc, mybir

    nc = bacc.Bacc(
        "TRN2",
        target_bir_lowering=False,
        debug=False,
        enable_asserts=False,
        num_devices=N_CORES,
    )
    f32 = mybir.dt.float32
    f16 = mybir.dt.float16

    G, C, pair_off = plan["G"], plan["C"], plan["pair_off"]
    CP, CPmax = plan["CP"], plan["CPmax"]
    NP = G // 2
    ocol = plan["ocol"]

    ind_d = nc.dram_tensor("ind", [128, plan["F"]], f16, kind="ExternalInput")
    od_d = nc.dram_tensor("od", [128, plan["Ftot_out"]], f16, kind="ExternalOutput")

    with tile.TileContext(nc) as tc:
        with (
            tc.tile_pool(name="inp", bufs=1) as ipool,
            tc.tile_pool(name="hbuf", bufs=4) as hpool,
            tc.tile_pool(name="obuf", bufs=6) as opool,
            tc.tile_pool(name="ph", bufs=2, space="PSUM") as phpool,
            tc.tile_pool(name="po", bufs=3, space="PSUM") as popool,
        ):
            # --- resident input tiles ---
            # pairs 0-1: separate [u] and [x] tiles (earliest compute
            # start); pairs >= 2: one [u | x] tile. vb and bias separate.
            NSPLIT = min(2, NP)
            u_slot, x_slot, tvb = [], [], []
            for p in range(NP):
                if p < NSPLIT:
                    tu = ipool.tile([128, KC * 128], f16, tag=f"u{p}", name=f"tu{p}")
                    tx = ipool.tile([128, KC * CP[p]], f16, tag=f"x{p}", name=f"tx{p}")
                    u_slot.append((tu, 0))
                    x_slot.append((tx, 0))
                else:
                    alen = KC * 128 + KC * CP[p]
                    tap = ipool.tile([128, alen], f16, tag=f"a{p}", name=f"ta{p}")
                    u_slot.append((tap, 0))
                    x_slot.append((tap, KC * 128))
                tvb.append(
                    ipool.tile([128, 1024], f16, tag=f"vb{p}", name=f"tvb{p}")
                )
            tbias = ipool.tile([128, OC * CPmax], f16, tag="bias", name="tbias")

            # --- load DMAs in first-use order, alternating HWDGE rings ---
            ring = [nc.sync, nc.scalar]
            rr = [0]

            def load(p, which):
                eng = ring[rr[0] % 2]
                rr[0] += 1
                u_o, x_o, vb_o = pair_off[p]
                if which == "u":
                    eng.dma_start(
                        out=u_slot[p][0][:, 0 : KC * 128],
                        in_=ind_d[:, u_o : u_o + KC * 128],
                    )
                elif which == "x":
                    t, o = x_slot[p]
                    eng.dma_start(
                        out=t[:, o : o + KC * CP[p]],
                        in_=ind_d[:, x_o : x_o + KC * CP[p]],
                    )
                elif which == "a":
                    n = KC * 128 + KC * CP[p]
                    eng.dma_start(out=u_slot[p][0][:], in_=ind_d[:, u_o : u_o + n])
                elif which == "vb":
                    eng.dma_start(out=tvb[p][:], in_=ind_d[:, vb_o : vb_o + 1024])
                else:  # bias
                    eng.dma_start(
                        out=tbias[:],
                        in_=ind_d[
                            :, pair_off[0][2] + 1024 : pair_off[0][2] + 1024 + OC * CPmax
                        ],
                    )

            # order: stage-1 inputs first (the tail is bound by the last
            # pair's x arrival), then bias, then the vb tables in use order
            load(0, "u")
            load(0, "x")
            if NP > 1:
                load(1, "u")
                load(1, "x")
            load(0, "vb")
            for p in range(2, NP):
                load(p, "a")
            load(0, "bias")
            for p in range(1, NP):
                load(p, "vb")

            bias_flat = tbias[:]

            # --- compute, software-pipelined: stage1(p) || stage2(p-1) ---
            hT_t = [None] * NP

            def stage1(p):
                C0, C1 = C[2 * p], C[2 * p + 1]
                ut, u_o = u_slot[p]
                xt, x_o = x_slot[p]
                # one full PSUM bank so slots stay bank-aligned
                ph = phpool.tile([128, 512], f32, tag="ph")
                for half, (Cg, xo) in enumerate(((C0, x_o), (C1, x_o + KC * C0))):
                    for k in range(KC):
                        nc.tensor.matmul(
                            ph[:, half * C0 : half * C0 + Cg],
                            lhsT=ut[:, u_o + k * 128 : u_o + (k + 1) * 128],
                            rhs=xt[:, xo + k * Cg : xo + (k + 1) * Cg],
                            start=(k == 0),
                            stop=(k == KC - 1),
                        )
                # hT layout [128, C0+C1]: lo tokens cols 0:C0 rows 0-63, hi
                # tokens cols C0:C0+C1 rows 64-127; off-quadrants zeroed so
                # the fused pair matmul picks each token's own expert.
                hT = hpool.tile([128, C0 + C1], f16, tag="hT")
                nc.gpsimd.memset(hT[64:128, 0:C0], 0.0)
                nc.gpsimd.memset(hT[0:64, C0 : C0 + C1], 0.0)
                nc.scalar.copy(hT[0:64, 0:C0], ph[0:64, 0:C0])
                nc.vector.tensor_copy(hT[64:128, C0 : C0 + C1], ph[64:128, C0 : C0 + C1])
                hT_t[p] = hT

            def stage2(p):
                cp = CP[p]
                vt, vb_o = tvb[p], 0
                hT = hT_t[p]
                bt = bias_flat.rearrange("q (c t) -> q c t", c=OC)
                # fused pair: stationary [128, 128] V chunk (rows 0-63 V_lo,
                # 64-127 V_hi), moving hT [128, cp] -> out^T [128, cp].
                # Split in two 4-chunk halves so the epilogue add and store
                # of half A overlap the matmuls of half B.
                for h in range(2):
                    po = popool.tile([128, 4, 256], f32, tag="po")
                    for i in range(4):
                        c = 4 * h + i
                        nc.tensor.matmul(
                            po[:, i, 0:cp],
                            lhsT=vt[:, vb_o + c * 128 : vb_o + (c + 1) * 128],
                            rhs=hT[:, 0:cp],
                            start=True,
                            stop=True,
                        )
                    o_sb = opool.tile([128, 4, cp], f16, tag="o")
                    nc.vector.tensor_add(
                        o_sb[:], po[:, :, 0:cp], bt[:, 4 * h : 4 * h + 4, 0:cp]
                    )
                    eng = ring[(2 * p + h) % 2]
                    eng.dma_start(
                        out=od_d[:, ocol[p] + 4 * h * cp : ocol[p] + (4 * h + 4) * cp],
                        in_=o_sb[:],
                    )

            for p in range(NP):
                stage1(p)
                if p > 0:
                    stage2(p - 1)
            stage2(NP - 1)

    nc.compile()
    return nc


def kernel(x, pids, U, V, bias):
    global LAST_EXEC_TIME_NS, LAST_RESULTS
    from concourse.bass_utils import run_bass_kernel_spmd

    x16 = np.asarray(x, dtype=np.float16)
    pids_np = np.asarray(pids).astype(np.int64)
    U16 = np.asarray(U, dtype=np.float16)
    V16 = np.asarray(V, dtype=np.float16)
    bias16 = np.asarray(bias, dtype=np.float16)

    N = x16.shape[0]
    P = U16.shape[0]

    core_groups = _route(pids_np, P)
    plan = _plan(core_groups)

    in_maps = [
        {"ind": _pack_core(core_groups[c], plan, x16, U16, V16, bias16)}
        for c in range(N_CORES)
    ]

    key = (plan["G"], plan["C"])
    if key not in _PROGRAM_CACHE:
        _PROGRAM_CACHE[key] = _build_program(plan)
    nc = _PROGRAM_CACHE[key]

    trace = os.environ.get("KERNEL_TRACE", "0") == "1"
    res = run_bass_kernel_spmd(nc, in_maps, list(range(N_CORES)), trace=trace)
    LAST_EXEC_TIME_NS = res.exec_time_ns
    LAST_RESULTS = res

    out = np.zeros((N, D_OUT), np.float32)
    for c in range(N_CORES):
        _unpack_core(res.results[c]["od"], core_groups[c], plan, out)
    return out
